# revision 33
# baseline (speedup 1.0000x reference)
"""MoE block (top-2 routed 3x3 conv experts) Trainium2 Bass kernel.

Strategy: data-parallel over batch, 2 samples per core on 8 cores.
The conv is linear in the kernel, so the top-2 expert kernels are
combined with the routing probabilities first (w_comb = sum_e p_e W_e),
then one 3x3 SAME conv per sample runs as matmuls.

Design:
- fp16 conv path (x, expert weights, output staging); fp32 gate math
  (the top-2 probability margins are ~1e-5 for this gate).
- x padded to [C, 130*130] on the HOST so every input DMA is one fully
  contiguous flat range per partition (per-row 256B segments sit below
  the 512B SDMA line-rate threshold and run at ~50 GB/s).
- Partitions 0-63 = padded channels; 64-127 = same data shifted +2
  elements, so a [128, 4x128] rhs read gives taps (dy,-1) on top and
  (dy,+1) on the bottom half; dx=0 taps are K=128 matmuls with zero
  weights on the bottom half (host-built zeros; keeping every conv
  matmul K=128 avoids PE reconfig stalls between row configurations).
- Residual folded into the center-tap expert weights on host
  (W_e[center] += I; routing probs sum to 1), so the post-conv op is a
  pure bias add.
- Column-tiled conv pairs: tile A (rows 8p..8p+4) accumulates in PSUM
  partitions 0:64 via PE column group 0 while tile B (8p+4..8p+8) runs
  concurrently in partitions 64:128 / column group 1. Two pairs share a
  [128, 2, 4, W] stage tile (ACT Identity+bias / DVE tensor_scalar_add
  posts) and two batched out DMAs with interleaved 4-row blocks.
- Pooled partials: ACT sums xpad[0:2QS) from the top copy, DVE sums
  xpad[2QS:FLAT) via the bottom copy, each in 4 ~2.1K-element pieces
  that pipeline with the chunk DMAs. Sample 1's pieces are dep-pinned
  behind sample 0's gate chain so the static scheduler cannot run them
  first (it otherwise delays wcomb by ~12us).

DMA queues: SP = top halves + expert weights A + even stage DMAs;
ACT = bottom halves (quarters 2,3 first, feeding pooled; 0,1 held
until the softmax exp has issued) + odd stage DMAs; gpsimd = gate
weights + expert weights B.
"""
import numpy as np
from contextlib import ExitStack

import concourse.bass as bass
import concourse.tile as tile
from concourse import bacc, mybir
from concourse.bass_utils import run_bass_kernel_spmd
from concourse.tile import add_dep_helper

F32 = mybir.dt.float32
F16 = mybir.dt.float16
AX = mybir.AxisListType
OP = mybir.AluOpType
ACTF = mybir.ActivationFunctionType

B, C, H, W, E, GH = 16, 64, 128, 128, 8, 16
NCORES = 8
SPB = B // NCORES          # samples per core
HP, WP = H + 2, W + 2      # 130
FLAT = HP * WP             # 16900
NB = H // 16               # 8 conv blocks (2 pairs = 16 output rows each)
QS = FLAT // 4             # 4225: flat quarter-chunk
HQ = QS // 2               # 2112: pooled piece size
BEND = FLAT - 2            # last valid element of the shifted bottom copy
GATE_SPLIT = 4             # emit sample-1 gate work after this many s0 blocks

_cache = {}


def _emit_sample_loads(nc, pools, s, XX, xs_ap, mid_sp=None, pooled_pieces=True):
    """Input DMAs + pooled partial pieces for sample s.

    SP queue: top quarters 0-3 (+ mid_sp between 1 and 2). ACT queue:
    bottom quarters 2,3 (pooled inputs) first, 0,1 late. ACT engine
    sums xpad[0:2QS) (top quarters 0,1); DVE sums xpad[2QS:FLAT) via
    the bottom copy (bottom dst [a:b) holds xpad[a+2:b+2)).
    Returns (pooled, piece_instructions, late_bot_dmas).
    """
    f = pools
    part = f["gate"].tile([128, 4], F32, tag="part", name=f"part{s}")

    def top(k):
        nc.sync.dma_start(
            XX[0:64, QS * k : QS * (k + 1)],
            xs_ap[s, :, QS * k : QS * (k + 1)],
        )

    def bot(k):
        a = max(QS * k - 2, 0)
        b = min(QS * (k + 1) - 2, BEND)
        return nc.scalar.dma_start(XX[64:128, a:b], xs_ap[s, :, a + 2 : b + 2])

    top(0)
    top(1)
    if mid_sp is not None:
        mid_sp()
    top(2)
    top(3)
    bot(2)
    bot(3)
    bot_dmas = [bot(0), bot(1)]

    pieces = []
    if not pooled_pieces:
        return None, pieces, bot_dmas

    def act_piece(a, b, col):
        scr = f["scratch"].tile([64, 2 * HQ + 8], F16, tag="scrA", name=f"scrA{s}_{col}")
        pieces.append(nc.scalar.activation(
            scr[:, 0 : b - a],
            XX[0:64, a:b],
            ACTF.Copy,
            accum_out=part[0:64, col : col + 1],
        ))

    def dve_piece(a, b, col):
        scr = f["scratch"].tile([128, 2 * HQ + 8], F16, tag="scrB", name=f"scrB{s}_{col}")
        pieces.append(nc.vector.tensor_scalar(
            scr[64:128, 0 : b - a],
            XX[64:128, a:b],
            0.0,
            0.0,
            OP.add,
            OP.add,
            accum_out=part[64:128, col : col + 1],
        ))

    act_piece(0, HQ, 0)                        # xpad[0:2112)
    act_piece(HQ, QS, 1)                       # xpad[2112:4225)
    act_piece(QS, QS + HQ, 2)                  # xpad[4225:6337)
    act_piece(QS + HQ, 2 * QS, 3)              # xpad[6337:8450)
    dve_piece(2 * QS - 2, 2 * QS - 2 + HQ, 0)  # xpad[8450:10562)
    dve_piece(2 * QS - 2 + HQ, 3 * QS - 2, 1)  # xpad[10562:12675)
    dve_piece(3 * QS - 2, 3 * QS - 2 + HQ, 2)  # xpad[12675:14787)
    dve_piece(3 * QS - 2 + HQ, BEND, 3)        # xpad[14787:FLAT)

    pooled = f["gate"].tile([128, 1], F32, tag="pooled", name=f"pooled{s}")
    nc.vector.tensor_reduce(pooled, part[:], axis=AX.X, op=OP.add)
    return pooled, pieces, bot_dmas


def _emit_pe_pooled(nc, pools, s, XX, wg1h_sb):
    """Gate hidden pre-activations via the (pre-conv idle) PE: 34 K=64
    matmuls accumulate wg1^T @ x over 512-element flat chunks into one
    PSUM tile [GH, 512], pipelining with the top-copy chunk DMAs. The
    1/(H*W) pooled scaling is folded into wg2 on the host.
    """
    hm = pools["gpsum"].tile([GH, 512], F32, tag="wbps", name=f"hm{s}", bufs=1)
    nmm = (FLAT + 511) // 512
    for k in range(nmm):
        a = 512 * k
        n = min(512, FLAT - a)
        nc.tensor.matmul(
            hm[:, 0:n],
            lhsT=wg1h_sb[:],
            rhs=XX[0:64, a : a + n],
            start=(k == 0),
            stop=(k == nmm - 1),
        )
    return hm


def _emit_sample_gate(nc, pools, s, pooled, consts, h_ext):
    """Gate MLP + softmax + top-2 + combined weights/bias for one sample.

    `pooled` is either ("pe", hm) — a [GH, 512] PSUM tile of hidden
    pre-activation partials to reduce — or ("vec", pooled128) — raw
    channel sums to send through the wg1 matmul.

    Uses exp-without-max-sub (logits are small) and folds the top-2 mask
    and renormalization:  w8 = (u>=m2)*u / (sum((u>=m2)*u) + sum(u)*1e-8)
    which equals the reference's normalized-probs formula exactly.
    Returns (wcombr, b_comb, exp_inst, tail_inst).
    """
    f = pools
    g = f["gate"]
    wg1x2_sb, bg1_sb, bg1hw_sb, wg2_sb, wg2hw_sb, bexp_sb, wps_sb, ones = consts
    n = lambda base: f"{base}{s}"

    kind, src = pooled
    if kind == "pe":
        h_red = g.tile([GH, 1], F32, tag="h_red", name=n("h_red"))
        nc.vector.tensor_reduce(h_red[:], src[:], axis=AX.X, op=OP.add)
        nc.vector.tensor_scalar(
            h_ext[0:GH, :], h_red[:], bg1hw_sb[:], 0.0, OP.add, OP.max
        )
        wg2_use = wg2hw_sb
    else:
        h_ps = f["gpsum"].tile([GH, 1], F32, tag="gps", name=n("h_ps"))
        nc.tensor.matmul(h_ps[:], lhsT=wg1x2_sb[:], rhs=src[:], start=True, stop=True)
        nc.vector.tensor_scalar(
            h_ext[0:GH, :], h_ps[:], bg1_sb[:], 0.0, OP.add, OP.max
        )
        wg2_use = wg2_sb

    lg_ps = f["gpsum"].tile([1, E], F32, tag="gps", name=n("lg_ps"))
    nc.tensor.matmul(lg_ps[:], lhsT=h_ext[:], rhs=wg2_use[:], start=True, stop=True)

    # u = exp(logits) (unnormalized softmax; |logits| is tiny, no max-sub)
    u = g.tile([1, E], F32, tag="u", name=n("u"))
    exp_inst = nc.scalar.activation(u[:], lg_ps[:], ACTF.Exp)
    usum = g.tile([1, 1], F32, tag="usum", name=n("usum"))
    nc.vector.tensor_reduce(usum[:], u[:], axis=AX.X, op=OP.add)
    # top-2: pm = (u < max)*u (valid since u>0), m2 = 2nd max, spv = (u>=m2)*u
    m1p = g.tile([1, 1], F32, tag="m1p", name=n("m1p"))
    nc.vector.tensor_reduce(m1p[:], u[:], axis=AX.X, op=OP.max)
    pm = g.tile([1, E], F32, tag="pm", name=n("pm"))
    nc.vector.scalar_tensor_tensor(pm[:], u[:], m1p[:], u[:], op0=OP.is_lt, op1=OP.mult)
    m2 = g.tile([1, 1], F32, tag="m2", name=n("m2"))
    nc.vector.tensor_reduce(m2[:], pm[:], axis=AX.X, op=OP.max)
    spv = g.tile([1, E], F32, tag="spv", name=n("spv"))
    nc.vector.scalar_tensor_tensor(spv[:], u[:], m2[:], u[:], op0=OP.is_ge, op1=OP.mult)
    dsum = g.tile([1, 1], F32, tag="dsum", name=n("dsum"))
    nc.vector.tensor_reduce(dsum[:], spv[:], axis=AX.X, op=OP.add)
    dd = g.tile([1, 1], F32, tag="dd", name=n("dd"))
    nc.vector.scalar_tensor_tensor(dd[:], usum[:], 1e-8, dsum[:], op0=OP.mult, op1=OP.add)
    rr = g.tile([1, 1], F32, tag="rr", name=n("rr"))
    nc.vector.reciprocal(rr[:], dd[:])
    w8 = g.tile([1, E], F32, tag="w8", name=n("w8"))
    nc.vector.tensor_scalar_mul(w8[:], spv[:], rr[:])

    # broadcast w8 down all 128 partitions: [128, E] = ones[1,128]^T @ w8[1,E]
    wb_ps = f["gpsum"].tile([128, E], F32, tag="wbps", name=n("wb_ps"), bufs=1)
    nc.tensor.matmul(wb_ps[:], lhsT=ones[:], rhs=w8[:], start=True, stop=True)
    wb128 = wb_ps

    # combined bias path (off critical path): b_comb128 = (b_exp
    # duplicated over both partition halves)^T @ w8^T
    w8c_ps = f["gpsum"].tile([E, 1], F32, tag="gps", name=n("w8c_ps"))
    nc.tensor.matmul(w8c_ps[:], lhsT=w8[:], rhs=ones[:, 0:1], start=True, stop=True)
    w8col = g.tile([E, 1], F32, tag="w8col", name=n("w8col"))
    nc.vector.tensor_copy(w8col[:], w8c_ps[:])
    bc_ps = f["gpsum"].tile([128, 1], F32, tag="gps2", name=n("bc_ps"))
    nc.tensor.matmul(bc_ps[:], lhsT=bexp_sb[:], rhs=w8col[:], start=True, stop=True)
    b_comb = g.tile([128, 1], F32, tag="b_comb", name=n("b_comb"))
    nc.vector.tensor_copy(b_comb[:], bc_ps[:])

    # combined conv weights: one fused MAC chain over [128, 6, C]
    # (slots 0-2 = paired dx taps, 3-5 = dx=0 taps w/ zero bottom rows;
    # slot 4 top half carries +I for the residual shortcut)
    wcomb = f["wcomb"].tile([128, 6, C], F32, tag="wcomb", name=n("wcomb"))
    nc.vector.tensor_scalar_mul(wcomb[:], wps_sb[:, 0], wb128[:, 0:1])
    for e in range(1, E):
        nc.vector.scalar_tensor_tensor(
            wcomb[:], wps_sb[:, e], wb128[:, e : e + 1], wcomb[:],
            op0=OP.mult, op1=OP.add,
        )
    wcombr = f["wcomb"].tile([128, 6, C], F16, tag="wcombr", name=n("wcombr"))
    tail = nc.vector.tensor_copy(wcombr[:], wcomb[:])
    return wcombr, b_comb, exp_inst, tail


def _emit_conv_blocks(nc, pools, s, XX, wcombr, b_comb, out_ap, blk_range):
    """Conv blocks (2 pairs x 8 output rows) for sample s.

    Within a pair, tile A (PSUM partitions 0:64, PE column group 0) and
    tile B (64:128, group 1) run their 6-matmul chains concurrently.
    """
    f = pools
    XX3 = XX[:, 0:FLAT].rearrange("p (r c) -> p r c", c=WP)
    # out rows as [blk, bank(2), half(2), row(4)]
    outv = out_ap[s].rearrange("c (blk i j r) w -> c blk i j r w", i=2, j=2, r=4)
    for blk in blk_range:
        stage = f["stage"].tile([128, 2, 4, W], F16, tag="stage", name=f"ost{s}_{blk}")
        for i in (0, 1):
            ps = f["cpsum"].tile([128, 4, W], F32, tag="cps", name=f"cps{s}_{blk}_{i}")
            r00 = 16 * blk + 8 * i
            for dyi in range(3):
                for half, r0 in ((0, r00), (64, r00 + 4)):
                    nc.tensor.matmul(
                        ps[half : half + 64],
                        lhsT=wcombr[:, dyi, :],
                        rhs=XX3[:, r0 + dyi : r0 + dyi + 4, 0:128],
                        start=(dyi == 0),
                        stop=False,
                    )
            for dyi in range(3):
                for half, r0 in ((0, r00), (64, r00 + 4)):
                    nc.tensor.matmul(
                        ps[half : half + 64],
                        lhsT=wcombr[:, 3 + dyi, :],
                        rhs=XX3[:, r0 + dyi : r0 + dyi + 4, 1:129],
                        start=False,
                        stop=(dyi == 2),
                    )
            if i == 0:
                nc.scalar.activation(
                    stage[:, 0], ps[:], ACTF.Identity, bias=b_comb[:, 0:1]
                )
            else:
                nc.vector.tensor_scalar_add(stage[:, 1], ps[:], b_comb[:, 0:1])
        # half j=0: banks' partitions 0:64 -> row-blocks (blk, 0/1, 0)
        nc.sync.dma_start(outv[:, blk, :, 0], stage[0:64])
        nc.scalar.dma_start(outv[:, blk, :, 1], stage[64:128])


def build_program():
    if "nc" in _cache:
        return _cache["nc"]
    nc = bacc.Bacc("TRN2", target_bir_lowering=False, debug=False, enable_asserts=False)
    xs_ap = nc.dram_tensor("xs", [SPB, C, FLAT], F16, kind="ExternalInput").ap()
    wpsA_d = nc.dram_tensor("wpsA", [128, E // 2, 6, C], F16, kind="ExternalInput").ap()
    wpsB_d = nc.dram_tensor("wpsB", [128, E // 2, 6, C], F16, kind="ExternalInput").ap()
    wg1_d = nc.dram_tensor("wg1", [128, GH], F32, kind="ExternalInput").ap()
    wg1h_d = nc.dram_tensor("wg1h", [64, GH], F16, kind="ExternalInput").ap()
    bg1_d = nc.dram_tensor("bg1", [GH, 1], F32, kind="ExternalInput").ap()
    bg1hw_d = nc.dram_tensor("bg1hw", [GH, 1], F32, kind="ExternalInput").ap()
    wg2_d = nc.dram_tensor("wg2", [GH + 1, E], F32, kind="ExternalInput").ap()
    wg2hw_d = nc.dram_tensor("wg2hw", [GH + 1, E], F32, kind="ExternalInput").ap()
    bexp_d = nc.dram_tensor("b_exp", [E, 128], F32, kind="ExternalInput").ap()
    out_ap = nc.dram_tensor("out", [SPB, C, H, W], F16, kind="ExternalOutput").ap()

    with tile.TileContext(nc) as tc, ExitStack() as ctx:
        pools = {
            "const": ctx.enter_context(tc.tile_pool(name="const", bufs=1)),
            "xx": ctx.enter_context(tc.tile_pool(name="xx", bufs=SPB)),
            "gate": ctx.enter_context(tc.tile_pool(name="gate", bufs=2)),
            "wcomb": ctx.enter_context(tc.tile_pool(name="wcomb", bufs=2)),
            "stage": ctx.enter_context(tc.tile_pool(name="stage", bufs=4)),
            "scratch": ctx.enter_context(tc.tile_pool(name="scratch", bufs=1)),
            "gpsum": ctx.enter_context(tc.tile_pool(name="gpsum", bufs=1, space="PSUM")),
            "cpsum": ctx.enter_context(tc.tile_pool(name="cpsum", bufs=5, space="PSUM")),
        }
        cp = pools["const"]
        XX0 = pools["xx"].tile([128, FLAT], F16, tag="XX", name="XX0")
        XX1 = pools["xx"].tile([128, FLAT], F16, tag="XX", name="XX1")
        # the shifted bottom copy leaves its last 2 elements unwritten;
        # they are read (x zero weights) by the last row's dx=0 matmuls
        # and NaN garbage would poison the accumulation
        nc.vector.memset(XX0[64:128, BEND:FLAT], 0.0)
        nc.vector.memset(XX1[64:128, BEND:FLAT], 0.0)
        ones = cp.tile([1, 128], F32)
        nc.gpsimd.memset(ones[:], 1.0)
        # prewarm the ACT exp table before the ACT lane fills with DMAs
        warm = cp.tile([1, 1], F32)
        nc.scalar.activation(warm[:], ones[:, 0:1], ACTF.Exp)
        # h_ext = [relu(...); 1.0] buffers: write the trailing 1.0 rows
        # once, off the gate critical path (SWDGE fixed cost ~1-2us)
        g = pools["gate"]
        h_exts = [g.tile([GH + 1, 1], F32, tag="h_ext", name=f"h_ext{s}") for s in (0, 1)]
        nc.gpsimd.dma_start(h_exts[0][GH : GH + 1, 0:1], ones[0:1, 0:1])
        nc.gpsimd.dma_start(h_exts[1][GH : GH + 1, 0:1], ones[0:1, 0:1])
        # tiny gate weights + expert weights B on the gpsimd SWDGE lane
        wg1x2_sb = cp.tile([128, GH], F32)
        nc.gpsimd.dma_start(wg1x2_sb[:], wg1_d[:])
        wg1h_sb = cp.tile([64, GH], F16)
        nc.gpsimd.dma_start(wg1h_sb[:], wg1h_d[:])
        bg1_sb = cp.tile([GH, 1], F32)
        nc.gpsimd.dma_start(bg1_sb[:], bg1_d[:])
        bg1hw_sb = cp.tile([GH, 1], F32)
        nc.gpsimd.dma_start(bg1hw_sb[:], bg1hw_d[:])
        wg2_sb = cp.tile([GH + 1, E], F32)
        nc.gpsimd.dma_start(wg2_sb[:], wg2_d[:])
        wg2hw_sb = cp.tile([GH + 1, E], F32)
        nc.gpsimd.dma_start(wg2hw_sb[:], wg2hw_d[:])
        bexp_sb = cp.tile([E, 128], F32)
        nc.gpsimd.dma_start(bexp_sb[:], bexp_d[:])
        wps_sb = cp.tile([128, E, 6, C], F16)
        nc.gpsimd.dma_start(wps_sb[:, E // 2 :], wpsB_d[:])

        def load_wpsA():
            nc.sync.dma_start(wps_sb[:, : E // 2], wpsA_d[:])

        pooled0, pieces0, bots0 = _emit_sample_loads(nc, pools, 0, XX0, xs_ap, mid_sp=load_wpsA, pooled_pieces=False)
        hm0 = _emit_pe_pooled(nc, pools, 0, XX0, wg1h_sb)
        consts = (wg1x2_sb, bg1_sb, bg1hw_sb, wg2_sb, wg2hw_sb, bexp_sb, wps_sb, ones)

        g0 = _emit_sample_gate(nc, pools, 0, ("pe", hm0), consts, h_exts[0])
        add_dep_helper(bots0[0].ins, g0[2].ins, sync=False,
                       reason="s0 late bottom DMAs after s0 softmax exp")

        pooled1, pieces1, bots1 = _emit_sample_loads(nc, pools, 1, XX1, xs_ap)
        # pin s1's late bottom DMAs behind s0's exp: a DMA waiting for a
        # completion-sem lane at the ACT engine FIFO head otherwise
        # blocks the exp (and everything behind it) for ~10us
        for bd in bots1:
            add_dep_helper(bd.ins, g0[2].ins, sync=False,
                           reason="s1 late bottom DMAs after s0 softmax exp")
        # pin s1's pooled pieces behind s0's gate chain: the static
        # scheduler otherwise interleaves them into the chain and delays
        # s0's combined weights by ~12us
        for p in pieces1:
            add_dep_helper(p.ins, g0[3].ins, sync=False,
                           reason="s1 pooled pieces after s0 wcomb chain")

        _emit_conv_blocks(nc, pools, 0, XX0, *g0[:2], out_ap, range(0, GATE_SPLIT))
        g1 = _emit_sample_gate(nc, pools, 1, ("vec", pooled1), consts, h_exts[1])
        _emit_conv_blocks(nc, pools, 0, XX0, *g0[:2], out_ap, range(GATE_SPLIT, NB))
        _emit_conv_blocks(nc, pools, 1, XX1, *g1[:2], out_ap, range(0, NB))

    nc.compile()
    _cache["nc"] = nc
    return nc


def host_prep(x, wg1, bg1, wg2, bg2, w_exp, b_exp):
    """Host-side layout prep + per-core sharding. Returns in_maps list."""
    x = np.asarray(x, dtype=np.float32).astype(np.float16)
    xpad = np.zeros((B, C, HP, WP), dtype=np.float16)
    xpad[:, :, 1:129, 1:129] = x
    xpad = xpad.reshape(B, C, FLAT)
    wg1 = np.asarray(wg1, dtype=np.float32)
    bg1 = np.asarray(bg1, dtype=np.float32).reshape(GH, 1)
    wg2 = np.asarray(wg2, dtype=np.float32)
    bg2 = np.asarray(bg2, dtype=np.float32).reshape(1, E)
    w_exp = np.asarray(w_exp, dtype=np.float32)
    b_exp = np.asarray(b_exp, dtype=np.float32)

    # w_exp [E, O, I, KH, KW] -> wt [I, E, KH, KW, O]
    wt = np.transpose(w_exp, (2, 0, 3, 4, 1)).copy()
    # residual shortcut: out += x == each expert's center tap += I
    # (routing probs sum to 1 up to the reference's 1e-8 epsilon)
    wt[:, :, 1, 1, :] += np.eye(C, dtype=np.float32)[:, None, :]
    # paired taps: top partitions = dx=-1, bottom = dx=+1
    wpair = np.concatenate([wt[:, :, :, 0, :], wt[:, :, :, 2, :]], axis=0)
    # single taps: dx=0 on top, zeros on bottom
    wsing = np.concatenate([wt[:, :, :, 1, :], np.zeros_like(wt[:, :, :, 1, :])], axis=0)
    # merged [128, E, 6, O]: slots 0-2 pairs, 3-5 singles
    wps = np.concatenate([wpair, wsing], axis=2).astype(np.float16)

    shared = {
        "wpsA": np.ascontiguousarray(wps[:, 0:4]),
        "wpsB": np.ascontiguousarray(wps[:, 4:8]),
        "wg1": np.ascontiguousarray(np.concatenate([wg1, wg1], axis=0) / (H * W)),
        "wg1h": np.ascontiguousarray(wg1.astype(np.float16)),
        "bg1": np.ascontiguousarray(bg1),
        "bg1hw": np.ascontiguousarray(bg1 * (H * W)),
        "wg2": np.ascontiguousarray(np.concatenate([wg2, bg2], axis=0)),
        "wg2hw": np.ascontiguousarray(np.concatenate([wg2 / (H * W), bg2], axis=0)),
        "b_exp": np.ascontiguousarray(np.concatenate([b_exp, b_exp], axis=1)),
    }
    return [
        {"xs": np.ascontiguousarray(xpad[SPB * k : SPB * (k + 1)]), **shared}
        for k in range(NCORES)
    ]


def kernel(x, wg1, bg1, wg2, bg2, w_exp, b_exp):
    nc = build_program()
    in_maps = host_prep(x, wg1, bg1, wg2, bg2, w_exp, b_exp)
    res = run_bass_kernel_spmd(nc, in_maps, list(range(NCORES)))
    return np.concatenate(
        [res.results[k]["out"].astype(np.float32) for k in range(NCORES)], axis=0
    )


# revision 34
# speedup vs baseline: 1.1467x; 1.1467x over previous
"""MoE block (top-2 routed 3x3 conv experts) Trainium2 Bass kernel.

Strategy: data-parallel over batch, 2 samples per core on 8 cores.
The conv is linear in the kernel, so the top-2 expert kernels are
combined with the routing probabilities first (w_comb = sum_e p_e W_e),
then one 3x3 SAME conv per sample runs as matmuls.

Design:
- fp16 conv path (x, expert weights, output staging); fp32 gate math
  (the top-2 probability margins are ~1e-5 for this gate).
- x padded to [C, 130*130] on the HOST so every input DMA is one fully
  contiguous flat range per partition (per-row 256B segments sit below
  the 512B SDMA line-rate threshold and run at ~50 GB/s).
- Partitions 0-63 = padded channels; 64-127 = same data shifted +2
  elements, so a [128, 4x128] rhs read gives taps (dy,-1) on top and
  (dy,+1) on the bottom half; dx=0 taps are K=128 matmuls with zero
  weights on the bottom half (host-built zeros; keeping every conv
  matmul K=128 avoids PE reconfig stalls between row configurations).
- Residual folded into the center-tap expert weights on host
  (W_e[center] += I; routing probs sum to 1), so the post-conv op is a
  pure bias add.
- Column-tiled conv pairs: tile A (rows 8p..8p+4) accumulates in PSUM
  partitions 0:64 via PE column group 0 while tile B (8p+4..8p+8) runs
  concurrently in partitions 64:128 / column group 1. Two pairs share a
  [128, 2, 4, W] stage tile (ACT Identity+bias / DVE tensor_scalar_add
  posts) and two batched out DMAs with interleaved 4-row blocks.
- Pooled partials: ACT sums xpad[0:2QS) from the top copy, DVE sums
  xpad[2QS:FLAT) via the bottom copy, each in 4 ~2.1K-element pieces
  that pipeline with the chunk DMAs. Sample 1's pieces are dep-pinned
  behind sample 0's gate chain so the static scheduler cannot run them
  first (it otherwise delays wcomb by ~12us).

DMA queues: SP = top halves + expert weights A + even stage DMAs;
ACT = bottom halves (quarters 2,3 first, feeding pooled; 0,1 held
until the softmax exp has issued) + odd stage DMAs; gpsimd = gate
weights + expert weights B.
"""
import numpy as np
from contextlib import ExitStack

import concourse.bass as bass
import concourse.tile as tile
from concourse import bacc, mybir
from concourse.bass_utils import run_bass_kernel_spmd
from concourse.tile import add_dep_helper

F32 = mybir.dt.float32
F16 = mybir.dt.float16
AX = mybir.AxisListType
OP = mybir.AluOpType
ACTF = mybir.ActivationFunctionType

B, C, H, W, E, GH = 16, 64, 128, 128, 8, 16
NCORES = 8
SPB = B // NCORES          # samples per core
HP, WP = H + 2, W + 2      # 130
FLAT = HP * WP             # 16900
NB = H // 16               # 8 conv blocks (2 pairs = 16 output rows each)
QS = FLAT // 4             # 4225: flat quarter-chunk
HQ = QS // 2               # 2112: pooled piece size
BEND = FLAT - 2            # last valid element of the shifted bottom copy
GATE_SPLIT = 4             # emit sample-1 gate work after this many s0 blocks

_cache = {}


def _emit_sample_loads(nc, pools, s, XX, xs_ap, mid_sp=None, pooled_pieces=True):
    """Input DMAs + pooled partial pieces for sample s.

    SP queue: top quarters 0-3 (+ mid_sp between 1 and 2). ACT queue:
    bottom quarters 2,3 (pooled inputs) first, 0,1 late. ACT engine
    sums xpad[0:2QS) (top quarters 0,1); DVE sums xpad[2QS:FLAT) via
    the bottom copy (bottom dst [a:b) holds xpad[a+2:b+2)).
    Returns (pooled, piece_instructions, late_bot_dmas).
    """
    f = pools
    part = f["gate"].tile([128, 4], F32, tag="part", name=f"part{s}")

    def top(k):
        nc.sync.dma_start(
            XX[0:64, QS * k : QS * (k + 1)],
            xs_ap[s, :, QS * k : QS * (k + 1)],
        )

    def bot(k):
        a = max(QS * k - 2, 0)
        b = min(QS * (k + 1) - 2, BEND)
        return nc.scalar.dma_start(XX[64:128, a:b], xs_ap[s, :, a + 2 : b + 2])

    top(0)
    top(1)
    if mid_sp is not None:
        mid_sp()
    top(2)
    top(3)
    bot(2)
    bot(3)
    bot_dmas = [bot(0), bot(1)]

    pieces = []
    if not pooled_pieces:
        return None, pieces, bot_dmas

    def act_piece(a, b, col):
        scr = f["scratch"].tile([64, 2 * HQ + 8], F16, tag="scrA", name=f"scrA{s}_{col}")
        pieces.append(nc.scalar.activation(
            scr[:, 0 : b - a],
            XX[0:64, a:b],
            ACTF.Copy,
            accum_out=part[0:64, col : col + 1],
        ))

    def dve_piece(a, b, col):
        scr = f["scratch"].tile([128, 2 * HQ + 8], F16, tag="scrB", name=f"scrB{s}_{col}")
        pieces.append(nc.vector.tensor_scalar(
            scr[64:128, 0 : b - a],
            XX[64:128, a:b],
            0.0,
            0.0,
            OP.add,
            OP.add,
            accum_out=part[64:128, col : col + 1],
        ))

    act_piece(0, HQ, 0)                        # xpad[0:2112)
    act_piece(HQ, QS, 1)                       # xpad[2112:4225)
    act_piece(QS, QS + HQ, 2)                  # xpad[4225:6337)
    act_piece(QS + HQ, 2 * QS, 3)              # xpad[6337:8450)
    dve_piece(2 * QS - 2, 2 * QS - 2 + HQ, 0)  # xpad[8450:10562)
    dve_piece(2 * QS - 2 + HQ, 3 * QS - 2, 1)  # xpad[10562:12675)
    dve_piece(3 * QS - 2, 3 * QS - 2 + HQ, 2)  # xpad[12675:14787)
    dve_piece(3 * QS - 2 + HQ, BEND, 3)        # xpad[14787:FLAT)

    pooled = f["gate"].tile([128, 1], F32, tag="pooled", name=f"pooled{s}")
    nc.vector.tensor_reduce(pooled, part[:], axis=AX.X, op=OP.add)
    return pooled, pieces, bot_dmas


def _emit_pe_pooled(nc, pools, s, XX, wg1h_sb):
    """Gate hidden pre-activations via the (pre-conv idle) PE: 34 K=64
    matmuls accumulate wg1^T @ x over 512-element flat chunks into one
    PSUM tile [GH, 512], pipelining with the top-copy chunk DMAs. The
    1/(H*W) pooled scaling is folded into wg2 on the host.
    """
    hm = pools["gpsum"].tile([GH, 512], F32, tag="wbps", name=f"hm{s}", bufs=1)
    nmm = (FLAT + 511) // 512
    for k in range(nmm):
        a = 512 * k
        n = min(512, FLAT - a)
        nc.tensor.matmul(
            hm[:, 0:n],
            lhsT=wg1h_sb[:],
            rhs=XX[0:64, a : a + n],
            start=(k == 0),
            stop=(k == nmm - 1),
        )
    return hm


def _emit_sample_gate(nc, pools, s, pooled, consts, h_ext):
    """Gate MLP + softmax + top-2 + combined weights/bias for one sample.

    `pooled` is either ("pe", hm) — a [GH, 512] PSUM tile of hidden
    pre-activation partials to reduce — or ("vec", pooled128) — raw
    channel sums to send through the wg1 matmul.

    Uses exp-without-max-sub (logits are small) and folds the top-2 mask
    and renormalization:  w8 = (u>=m2)*u / (sum((u>=m2)*u) + sum(u)*1e-8)
    which equals the reference's normalized-probs formula exactly.
    Returns (wcombr, b_comb, exp_inst, tail_inst).
    """
    f = pools
    g = f["gate"]
    wg1x2_sb, bg1_sb, bg1hw_sb, wg2_sb, wg2hw_sb, bexp_sb, wps_sb, ones = consts
    n = lambda base: f"{base}{s}"

    kind, src = pooled
    if kind == "pe":
        h_red = g.tile([GH, 1], F32, tag="h_red", name=n("h_red"))
        nc.vector.tensor_reduce(h_red[:], src[:], axis=AX.X, op=OP.add)
        nc.vector.tensor_scalar(
            h_ext[0:GH, :], h_red[:], bg1hw_sb[:], 0.0, OP.add, OP.max
        )
        wg2_use = wg2hw_sb
    else:
        h_ps = f["gpsum"].tile([GH, 1], F32, tag="gps", name=n("h_ps"))
        nc.tensor.matmul(h_ps[:], lhsT=wg1x2_sb[:], rhs=src[:], start=True, stop=True)
        nc.vector.tensor_scalar(
            h_ext[0:GH, :], h_ps[:], bg1_sb[:], 0.0, OP.add, OP.max
        )
        wg2_use = wg2_sb

    lg_ps = f["gpsum"].tile([1, E], F32, tag="gps", name=n("lg_ps"))
    nc.tensor.matmul(lg_ps[:], lhsT=h_ext[:], rhs=wg2_use[:], start=True, stop=True)

    # u = exp(logits) (unnormalized softmax; |logits| is tiny, no max-sub)
    u = g.tile([1, E], F32, tag="u", name=n("u"))
    exp_inst = nc.scalar.activation(u[:], lg_ps[:], ACTF.Exp)
    usum = g.tile([1, 1], F32, tag="usum", name=n("usum"))
    nc.vector.tensor_reduce(usum[:], u[:], axis=AX.X, op=OP.add)
    # top-2: pm = (u < max)*u (valid since u>0), m2 = 2nd max, spv = (u>=m2)*u
    m1p = g.tile([1, 1], F32, tag="m1p", name=n("m1p"))
    nc.vector.tensor_reduce(m1p[:], u[:], axis=AX.X, op=OP.max)
    pm = g.tile([1, E], F32, tag="pm", name=n("pm"))
    nc.vector.scalar_tensor_tensor(pm[:], u[:], m1p[:], u[:], op0=OP.is_lt, op1=OP.mult)
    m2 = g.tile([1, 1], F32, tag="m2", name=n("m2"))
    nc.vector.tensor_reduce(m2[:], pm[:], axis=AX.X, op=OP.max)
    spv = g.tile([1, E], F32, tag="spv", name=n("spv"))
    nc.vector.scalar_tensor_tensor(spv[:], u[:], m2[:], u[:], op0=OP.is_ge, op1=OP.mult)
    dsum = g.tile([1, 1], F32, tag="dsum", name=n("dsum"))
    nc.vector.tensor_reduce(dsum[:], spv[:], axis=AX.X, op=OP.add)
    dd = g.tile([1, 1], F32, tag="dd", name=n("dd"))
    nc.vector.scalar_tensor_tensor(dd[:], usum[:], 1e-8, dsum[:], op0=OP.mult, op1=OP.add)
    rr = g.tile([1, 1], F32, tag="rr", name=n("rr"))
    nc.vector.reciprocal(rr[:], dd[:])
    w8 = g.tile([1, E], F32, tag="w8", name=n("w8"))
    nc.vector.tensor_scalar_mul(w8[:], spv[:], rr[:])

    # broadcast w8 down all 128 partitions: [128, E] = ones[1,128]^T @ w8[1,E]
    wb_ps = f["gpsum"].tile([128, E], F32, tag="wbps", name=n("wb_ps"), bufs=1)
    nc.tensor.matmul(wb_ps[:], lhsT=ones[:], rhs=w8[:], start=True, stop=True)
    wb128 = wb_ps

    # combined bias path (off critical path): b_comb128 = (b_exp
    # duplicated over both partition halves)^T @ w8^T
    w8c_ps = f["gpsum"].tile([E, 1], F32, tag="gps", name=n("w8c_ps"))
    nc.tensor.matmul(w8c_ps[:], lhsT=w8[:], rhs=ones[:, 0:1], start=True, stop=True)
    w8col = g.tile([E, 1], F32, tag="w8col", name=n("w8col"))
    nc.vector.tensor_copy(w8col[:], w8c_ps[:])
    bc_ps = f["gpsum"].tile([128, 1], F32, tag="gps2", name=n("bc_ps"))
    nc.tensor.matmul(bc_ps[:], lhsT=bexp_sb[:], rhs=w8col[:], start=True, stop=True)
    b_comb = g.tile([128, 1], F32, tag="b_comb", name=n("b_comb"))
    nc.vector.tensor_copy(b_comb[:], bc_ps[:])

    # combined conv weights: one fused MAC chain over [128, 6, C]
    # (slots 0-2 = paired dx taps, 3-5 = dx=0 taps w/ zero bottom rows;
    # slot 4 top half carries +I for the residual shortcut)
    wcomb = f["wcomb"].tile([128, 6, C], F32, tag="wcomb", name=n("wcomb"))
    nc.vector.tensor_scalar_mul(wcomb[:], wps_sb[:, 0], wb128[:, 0:1])
    for e in range(1, E):
        nc.vector.scalar_tensor_tensor(
            wcomb[:], wps_sb[:, e], wb128[:, e : e + 1], wcomb[:],
            op0=OP.mult, op1=OP.add,
        )
    wcombr = f["wcomb"].tile([128, 6, C], F16, tag="wcombr", name=n("wcombr"))
    tail = nc.vector.tensor_copy(wcombr[:], wcomb[:])
    return wcombr, b_comb, exp_inst, tail


def _emit_conv_blocks(nc, pools, s, XX, wcombr, b_comb, out_ap, blk_range):
    """Conv blocks (2 pairs x 8 output rows) for sample s.

    Within a pair, tile A (PSUM partitions 0:64, PE column group 0) and
    tile B (64:128, group 1) run their 6-matmul chains concurrently.
    """
    f = pools
    XX3 = XX[:, 0:FLAT].rearrange("p (r c) -> p r c", c=WP)
    # out rows as [blk, bank(2), half(2), row(4)]
    outv = out_ap[s].rearrange("c (blk i j r) w -> c blk i j r w", i=2, j=2, r=4)
    for blk in blk_range:
        stage = f["stage"].tile([128, 2, 4, W], F16, tag="stage", name=f"ost{s}_{blk}")
        for i in (0, 1):
            ps = f["cpsum"].tile([128, 4, W], F32, tag="cps", name=f"cps{s}_{blk}_{i}")
            r00 = 16 * blk + 8 * i
            for dyi in range(3):
                for half, r0 in ((0, r00), (64, r00 + 4)):
                    nc.tensor.matmul(
                        ps[half : half + 64],
                        lhsT=wcombr[:, dyi, :],
                        rhs=XX3[:, r0 + dyi : r0 + dyi + 4, 0:128],
                        start=(dyi == 0),
                        stop=False,
                    )
            for dyi in range(3):
                for half, r0 in ((0, r00), (64, r00 + 4)):
                    nc.tensor.matmul(
                        ps[half : half + 64],
                        lhsT=wcombr[:, 3 + dyi, :],
                        rhs=XX3[:, r0 + dyi : r0 + dyi + 4, 1:129],
                        start=False,
                        stop=(dyi == 2),
                    )
            if i == 0:
                nc.scalar.activation(
                    stage[:, 0], ps[:], ACTF.Identity, bias=b_comb[:, 0:1]
                )
            else:
                nc.vector.tensor_scalar_add(stage[:, 1], ps[:], b_comb[:, 0:1])
        # half j=0: banks' partitions 0:64 -> row-blocks (blk, 0/1, 0)
        nc.sync.dma_start(outv[:, blk, :, 0], stage[0:64])
        nc.scalar.dma_start(outv[:, blk, :, 1], stage[64:128])


def build_program():
    if "nc" in _cache:
        return _cache["nc"]
    nc = bacc.Bacc("TRN2", target_bir_lowering=False, debug=False, enable_asserts=False)
    xs_ap = nc.dram_tensor("xs", [SPB, C, FLAT], F16, kind="ExternalInput").ap()
    wpsA_d = nc.dram_tensor("wpsA", [128, E // 2, 6, C], F16, kind="ExternalInput").ap()
    wpsB_d = nc.dram_tensor("wpsB", [128, E // 2, 6, C], F16, kind="ExternalInput").ap()
    wg1_d = nc.dram_tensor("wg1", [128, GH], F32, kind="ExternalInput").ap()
    wg1h_d = nc.dram_tensor("wg1h", [64, GH], F16, kind="ExternalInput").ap()
    bg1_d = nc.dram_tensor("bg1", [GH, 1], F32, kind="ExternalInput").ap()
    bg1hw_d = nc.dram_tensor("bg1hw", [GH, 1], F32, kind="ExternalInput").ap()
    wg2_d = nc.dram_tensor("wg2", [GH + 1, E], F32, kind="ExternalInput").ap()
    wg2hw_d = nc.dram_tensor("wg2hw", [GH + 1, E], F32, kind="ExternalInput").ap()
    bexp_d = nc.dram_tensor("b_exp", [E, 128], F32, kind="ExternalInput").ap()
    out_ap = nc.dram_tensor("out", [SPB, C, H, W], F16, kind="ExternalOutput").ap()

    with tile.TileContext(nc) as tc, ExitStack() as ctx:
        pools = {
            "const": ctx.enter_context(tc.tile_pool(name="const", bufs=1)),
            "xx": ctx.enter_context(tc.tile_pool(name="xx", bufs=SPB)),
            "gate": ctx.enter_context(tc.tile_pool(name="gate", bufs=2)),
            "wcomb": ctx.enter_context(tc.tile_pool(name="wcomb", bufs=2)),
            "stage": ctx.enter_context(tc.tile_pool(name="stage", bufs=4)),
            "scratch": ctx.enter_context(tc.tile_pool(name="scratch", bufs=1)),
            "gpsum": ctx.enter_context(tc.tile_pool(name="gpsum", bufs=1, space="PSUM")),
            "cpsum": ctx.enter_context(tc.tile_pool(name="cpsum", bufs=5, space="PSUM")),
        }
        cp = pools["const"]
        XX0 = pools["xx"].tile([128, FLAT], F16, tag="XX", name="XX0")
        XX1 = pools["xx"].tile([128, FLAT], F16, tag="XX", name="XX1")
        # the shifted bottom copy leaves its last 2 elements unwritten;
        # they are read (x zero weights) by the last row's dx=0 matmuls
        # and NaN garbage would poison the accumulation
        nc.vector.memset(XX0[64:128, BEND:FLAT], 0.0)
        nc.vector.memset(XX1[64:128, BEND:FLAT], 0.0)
        ones = cp.tile([1, 128], F32)
        nc.gpsimd.memset(ones[:], 1.0)
        # prewarm the ACT exp table before the ACT lane fills with DMAs
        warm = cp.tile([1, 1], F32)
        nc.scalar.activation(warm[:], ones[:, 0:1], ACTF.Exp)
        # h_ext = [relu(...); 1.0] buffers: write the trailing 1.0 rows
        # once, off the gate critical path (SWDGE fixed cost ~1-2us)
        g = pools["gate"]
        h_exts = [g.tile([GH + 1, 1], F32, tag="h_ext", name=f"h_ext{s}") for s in (0, 1)]
        nc.gpsimd.dma_start(h_exts[0][GH : GH + 1, 0:1], ones[0:1, 0:1])
        nc.gpsimd.dma_start(h_exts[1][GH : GH + 1, 0:1], ones[0:1, 0:1])
        # tiny gate weights + expert weights B on the gpsimd SWDGE lane
        wg1x2_sb = cp.tile([128, GH], F32)
        nc.gpsimd.dma_start(wg1x2_sb[:], wg1_d[:])
        bg1_sb = cp.tile([GH, 1], F32)
        nc.gpsimd.dma_start(bg1_sb[:], bg1_d[:])
        wg2_sb = cp.tile([GH + 1, E], F32)
        nc.gpsimd.dma_start(wg2_sb[:], wg2_d[:])
        bexp_sb = cp.tile([E, 128], F32)
        nc.gpsimd.dma_start(bexp_sb[:], bexp_d[:])
        wps_sb = cp.tile([128, E, 6, C], F16)
        nc.gpsimd.dma_start(wps_sb[:, E // 2 :], wpsB_d[:])

        def load_wpsA():
            nc.sync.dma_start(wps_sb[:, : E // 2], wpsA_d[:])

        pooled0, pieces0, bots0 = _emit_sample_loads(nc, pools, 0, XX0, xs_ap, mid_sp=load_wpsA)
        consts = (wg1x2_sb, bg1_sb, None, wg2_sb, None, bexp_sb, wps_sb, ones)

        g0 = _emit_sample_gate(nc, pools, 0, ("vec", pooled0), consts, h_exts[0])
        add_dep_helper(bots0[0].ins, g0[2].ins, sync=False,
                       reason="s0 late bottom DMAs after s0 softmax exp")

        pooled1, pieces1, bots1 = _emit_sample_loads(nc, pools, 1, XX1, xs_ap)
        # pin s1's late bottom DMAs behind s0's exp: a DMA waiting for a
        # completion-sem lane at the ACT engine FIFO head otherwise
        # blocks the exp (and everything behind it) for ~10us
        for bd in bots1:
            add_dep_helper(bd.ins, g0[2].ins, sync=False,
                           reason="s1 late bottom DMAs after s0 softmax exp")
        # pin s1's pooled pieces behind s0's gate chain: the static
        # scheduler otherwise interleaves them into the chain and delays
        # s0's combined weights by ~12us
        for p in pieces1:
            add_dep_helper(p.ins, g0[3].ins, sync=False,
                           reason="s1 pooled pieces after s0 wcomb chain")

        _emit_conv_blocks(nc, pools, 0, XX0, *g0[:2], out_ap, range(0, GATE_SPLIT))
        g1 = _emit_sample_gate(nc, pools, 1, ("vec", pooled1), consts, h_exts[1])
        _emit_conv_blocks(nc, pools, 0, XX0, *g0[:2], out_ap, range(GATE_SPLIT, NB))
        _emit_conv_blocks(nc, pools, 1, XX1, *g1[:2], out_ap, range(0, NB))

    nc.compile()
    _cache["nc"] = nc
    return nc


def host_prep(x, wg1, bg1, wg2, bg2, w_exp, b_exp):
    """Host-side layout prep + per-core sharding. Returns in_maps list."""
    x = np.asarray(x, dtype=np.float32).astype(np.float16)
    xpad = np.zeros((B, C, HP, WP), dtype=np.float16)
    xpad[:, :, 1:129, 1:129] = x
    xpad = xpad.reshape(B, C, FLAT)
    wg1 = np.asarray(wg1, dtype=np.float32)
    bg1 = np.asarray(bg1, dtype=np.float32).reshape(GH, 1)
    wg2 = np.asarray(wg2, dtype=np.float32)
    bg2 = np.asarray(bg2, dtype=np.float32).reshape(1, E)
    w_exp = np.asarray(w_exp, dtype=np.float32)
    b_exp = np.asarray(b_exp, dtype=np.float32)

    # w_exp [E, O, I, KH, KW] -> wt [I, E, KH, KW, O]
    wt = np.transpose(w_exp, (2, 0, 3, 4, 1)).copy()
    # residual shortcut: out += x == each expert's center tap += I
    # (routing probs sum to 1 up to the reference's 1e-8 epsilon)
    wt[:, :, 1, 1, :] += np.eye(C, dtype=np.float32)[:, None, :]
    # paired taps: top partitions = dx=-1, bottom = dx=+1
    wpair = np.concatenate([wt[:, :, :, 0, :], wt[:, :, :, 2, :]], axis=0)
    # single taps: dx=0 on top, zeros on bottom
    wsing = np.concatenate([wt[:, :, :, 1, :], np.zeros_like(wt[:, :, :, 1, :])], axis=0)
    # merged [128, E, 6, O]: slots 0-2 pairs, 3-5 singles
    wps = np.concatenate([wpair, wsing], axis=2).astype(np.float16)

    shared = {
        "wpsA": np.ascontiguousarray(wps[:, 0:4]),
        "wpsB": np.ascontiguousarray(wps[:, 4:8]),
        "wg1": np.ascontiguousarray(np.concatenate([wg1, wg1], axis=0) / (H * W)),
        "wg1h": np.ascontiguousarray(wg1.astype(np.float16)),
        "bg1": np.ascontiguousarray(bg1),
        "bg1hw": np.ascontiguousarray(bg1 * (H * W)),
        "wg2": np.ascontiguousarray(np.concatenate([wg2, bg2], axis=0)),
        "wg2hw": np.ascontiguousarray(np.concatenate([wg2 / (H * W), bg2], axis=0)),
        "b_exp": np.ascontiguousarray(np.concatenate([b_exp, b_exp], axis=1)),
    }
    return [
        {"xs": np.ascontiguousarray(xpad[SPB * k : SPB * (k + 1)]), **shared}
        for k in range(NCORES)
    ]


def kernel(x, wg1, bg1, wg2, bg2, w_exp, b_exp):
    nc = build_program()
    in_maps = host_prep(x, wg1, bg1, wg2, bg2, w_exp, b_exp)
    res = run_bass_kernel_spmd(nc, in_maps, list(range(NCORES)))
    return np.concatenate(
        [res.results[k]["out"].astype(np.float32) for k in range(NCORES)], axis=0
    )
